# revision 4
# baseline (speedup 1.0000x reference)
"""PodDeepONet kernel for 8 Trainium2 NeuronCores.

Reference computes:
    h = tanh chain: noise[4096,128] -> W1..W4 (width 32) -> h4[4096,32]
    out_b  = h4 @ W5 + b5                      [4096,256]
    out_t  = coords @ Wc + bc                  [8192,256]
    out_bias = (coords @ Wm + bm)[:,0]         [8192]
    g = out_b @ out_t.T + out_bias[None,:]     [4096,8192] -> reshape(-1)

Since out_t / out_bias are affine in the 2-D coords, the K=256 GEMM
collapses algebraically:
    g[i,j] = a0[i]*x[j] + a1[i]*y[j] + q[i]
with   [a0,a1,q][i] = [h4[i],1] @ M1,   M1[33,3] folded on host from
W5,b5,Wc,bc,Wm,bm (float64).  The rank-3 product is evaluated on device
as a K=8 bf16 matmul using hi/lo splitting of both operands (error
~1e-5 rel, 1 cycle/row on the PE).

Sharding: data-parallel over the noise batch, 512 rows/core, 8 cores.
Each core runs the tiny MLP on its slice and writes a disjoint
[512,8192] f32 output slab.
"""

import numpy as np
import ml_dtypes

B = 4096
N = 8192
NF = 128
W = 32
NCORES = 8
BC = B // NCORES  # 512 rows per core

BF16 = ml_dtypes.bfloat16

_compiled = None


def _hi_lo(x32):
    hi = x32.astype(BF16)
    lo = (x32 - hi.astype(np.float32)).astype(BF16)
    return hi, lo


def _build_program():
    import concourse.bacc as bacc
    import concourse.tile as tile
    from concourse import mybir

    f32 = mybir.dt.float32
    bf16 = mybir.dt.bfloat16
    Tanh = mybir.ActivationFunctionType.Tanh

    nc = bacc.Bacc("TRN2", target_bir_lowering=False, debug=False,
                   enable_asserts=False, num_devices=NCORES)

    noiseT_d = nc.dram_tensor("noiseT", [NF, BC], f32, kind="ExternalInput").ap()
    w1_d = nc.dram_tensor("w1", [NF, W], f32, kind="ExternalInput").ap()
    w2_d = nc.dram_tensor("w2", [W, W], f32, kind="ExternalInput").ap()
    w3_d = nc.dram_tensor("w3", [W, W], f32, kind="ExternalInput").ap()
    w4_d = nc.dram_tensor("w4", [W, W], f32, kind="ExternalInput").ap()
    bmlp_d = nc.dram_tensor("bmlp", [W, 4], f32, kind="ExternalInput").ap()
    m1_d = nc.dram_tensor("m1", [W + 1, 3], f32, kind="ExternalInput").ap()
    rhs_d = nc.dram_tensor("rhs_aug", [8, N], bf16, kind="ExternalInput").ap()
    g_d = nc.dram_tensor("g", [BC, N], f32, kind="ExternalOutput").ap()

    import concourse.bass as bass

    with tile.TileContext(nc) as tc:
        with (
            tc.tile_pool(name="const", bufs=1) as cpool,
            tc.tile_pool(name="psum_prep", bufs=2, space="PSUM") as pp,
            tc.tile_pool(name="psum_main", bufs=4, space="PSUM") as pm,
            tc.tile_pool(name="gbuf", bufs=2) as gpool,
        ):
            noiseT = cpool.tile([NF, BC], f32)
            w1 = cpool.tile([NF, W], f32)
            w2 = cpool.tile([W, W], f32)
            w3 = cpool.tile([W, W], f32)
            w4 = cpool.tile([W, W], f32)
            bmlp = cpool.tile([W, 4], f32)
            m1 = cpool.tile([W + 1, 3], f32)
            rhs_aug = cpool.tile([8, N], bf16)

            nc.sync.dma_start(noiseT[:], noiseT_d[:])
            nc.sync.dma_start(w1[:], w1_d[:])
            nc.sync.dma_start(w2[:], w2_d[:])
            nc.sync.dma_start(w3[:], w3_d[:])
            nc.sync.dma_start(w4[:], w4_d[:])
            nc.sync.dma_start(bmlp[:], bmlp_d[:])
            nc.sync.dma_start(m1[:], m1_d[:])
            nc.gpsimd.dma_start(rhs_aug[:], rhs_d[:])

            h4aug = cpool.tile([W + 1, BC], f32)
            nc.gpsimd.memset(h4aug[W:W + 1, :], 1.0)

            h1 = cpool.tile([W, BC], f32)
            h2 = cpool.tile([W, BC], f32)
            h3 = cpool.tile([W, BC], f32)

            ps = pp.tile([W, BC], f32)
            nc.tensor.matmul(ps[:], w1[:], noiseT[:], start=True, stop=True)
            nc.scalar.activation(h1[:], ps[:], Tanh, bias=bmlp[:, 0:1])

            ps = pp.tile([W, BC], f32)
            nc.tensor.matmul(ps[:], w2[:], h1[:], start=True, stop=True)
            nc.scalar.activation(h2[:], ps[:], Tanh, bias=bmlp[:, 1:2])

            ps = pp.tile([W, BC], f32)
            nc.tensor.matmul(ps[:], w3[:], h2[:], start=True, stop=True)
            nc.scalar.activation(h3[:], ps[:], Tanh, bias=bmlp[:, 2:3])

            ps = pp.tile([W, BC], f32)
            nc.tensor.matmul(ps[:], w4[:], h3[:], start=True, stop=True)
            nc.scalar.activation(h4aug[0:W, :], ps[:], Tanh, bias=bmlp[:, 3:4])

            # Aq[3, BC] = M1.T @ [h4; 1]: rows a0, a1, q
            psA = pp.tile([3, BC], f32)
            nc.tensor.matmul(psA[:], m1[:], h4aug[:], start=True, stop=True)

            aq = cpool.tile([3, BC], f32)
            nc.vector.tensor_copy(aq[:], psA[:])

            # lhsT_aug rows: [a0h a1h qh a0l a1l ql a0h a1h]
            # compute hi/lo in partition-base-0 tiles (engine partition
            # access must be 32-aligned), then place rows by SBUF->SBUF
            # DMA which has no partition-base restriction.
            lhsT = cpool.tile([8, BC], bf16)
            hi_bf = cpool.tile([3, BC], bf16)
            lo_bf = cpool.tile([3, BC], bf16)
            hi32 = cpool.tile([3, BC], f32)
            lo32 = cpool.tile([3, BC], f32)
            nc.vector.tensor_copy(hi_bf[:], aq[:])
            nc.vector.tensor_copy(hi32[:], hi_bf[:])
            nc.vector.tensor_sub(lo32[:], aq[:], hi32[:])
            nc.vector.tensor_copy(lo_bf[:], lo32[:])
            nc.sync.dma_start(lhsT[0:3, :], hi_bf[:])
            nc.sync.dma_start(lhsT[3:6, :], lo_bf[:])
            nc.sync.dma_start(lhsT[6:8, :], hi_bf[0:2, :])

            # main loop: 4 row-blocks x 16 col-chunks of 512
            NB = BC // 128       # 4
            NCHUNK = N // 512    # 16
            for rb in range(NB):
                g_row = gpool.tile([128, N], f32)
                lt = lhsT[:, bass.ts(rb, 128)]
                for cc in range(NCHUNK):
                    psg = pm.tile([128, 512], f32)
                    nc.tensor.matmul(psg[:], lt, rhs_aug[:, bass.ts(cc, 512)],
                                     start=True, stop=True)
                    dst = g_row[:, bass.ts(cc, 512)]
                    if cc % 2 == 0:
                        nc.vector.tensor_copy(dst, psg[:])
                    else:
                        nc.scalar.copy(dst, psg[:])
                nc.sync.dma_start(g_d[bass.ts(rb, 128), :], g_row[:])

    nc.compile()
    return nc


def _get_program():
    global _compiled
    if _compiled is None:
        _compiled = _build_program()
    return _compiled


def kernel(**inputs):
    return run(inputs)[0]


def run(inputs, trace=False, trace_cores=None):
    noise = inputs["noise"].astype(np.float32, copy=False)
    coords = inputs["coordinates"].astype(np.float32, copy=False)
    W1 = inputs["W1"]; b1 = inputs["b1"]
    W2 = inputs["W2"]; b2 = inputs["b2"]
    W3 = inputs["W3"]; b3 = inputs["b3"]
    W4 = inputs["W4"]; b4 = inputs["b4"]
    W5 = inputs["W5"].astype(np.float64)
    b5 = inputs["b5"].astype(np.float64)
    Wc = inputs["Wc"].astype(np.float64)
    bc = inputs["bc"].astype(np.float64)
    Wm = inputs["Wm"].astype(np.float64)
    bm = inputs["bm"].astype(np.float64)

    # fold trunk + bias affine maps into M1 [33,3]
    M1 = np.zeros((W + 1, 3), np.float32)
    M1[:W, 0:2] = (W5 @ Wc.T).astype(np.float32)
    M1[W, 0:2] = (Wc @ b5 + Wm[:, 0]).astype(np.float32)
    M1[:W, 2] = (W5 @ bc).astype(np.float32)
    M1[W, 2] = np.float32(b5 @ bc + bm[0])

    bmlp = np.stack([b1, b2, b3, b4], axis=1).astype(np.float32)

    x, y = coords[:, 0], coords[:, 1]
    xh, xl = _hi_lo(x)
    yh, yl = _hi_lo(y)
    ones = np.ones(N, BF16)
    rhs_aug = np.ascontiguousarray(
        np.stack([xh, yh, ones, xh, yh, ones, xl, yl]))

    common = {
        "w1": np.ascontiguousarray(W1.astype(np.float32)),
        "w2": np.ascontiguousarray(W2.astype(np.float32)),
        "w3": np.ascontiguousarray(W3.astype(np.float32)),
        "w4": np.ascontiguousarray(W4.astype(np.float32)),
        "bmlp": bmlp,
        "m1": M1,
        "rhs_aug": rhs_aug,
    }
    in_maps = []
    for c in range(NCORES):
        m = dict(common)
        m["noiseT"] = np.ascontiguousarray(noise[c * BC:(c + 1) * BC].T)
        in_maps.append(m)

    from concourse.bass_utils import run_bass_kernel_spmd

    nc = _get_program()
    kw = {}
    if trace:
        kw["trace"] = True
        if trace_cores is not None:
            kw["trace_cores"] = trace_cores
    res = run_bass_kernel_spmd(nc, in_maps, list(range(NCORES)), **kw)
    out = np.concatenate([res.results[c]["g"] for c in range(NCORES)], axis=0)
    return out.reshape(-1), getattr(res, "exec_time_ns", None)


# revision 9
# speedup vs baseline: 65475.3022x; 65475.3022x over previous
"""PodDeepONet kernel for 8 Trainium2 NeuronCores.

Reference computes:
    h = tanh chain: noise[4096,128] -> W1..W4 (width 32) -> h4[4096,32]
    out_b  = h4 @ W5 + b5                      [4096,256]
    out_t  = coords @ Wc + bc                  [8192,256]
    out_bias = (coords @ Wm + bm)[:,0]         [8192]
    g = out_b @ out_t.T + out_bias[None,:]     [4096,8192] -> reshape(-1)

Since out_t / out_bias are affine in the 2-D coords, the K=256 GEMM
collapses algebraically:
    g[i,j] = a0[i]*x[j] + a1[i]*y[j] + q[i]
with   [a0,a1,q][i] = [h4[i],1] @ M1,   M1[33,3] folded on host from
W5,b5,Wc,bc,Wm,bm (float64).  The rank-3 product is evaluated on device
as a K=8 bf16 matmul using hi/lo splitting of both operands (error
~1e-5 rel, 1 cycle/row on the PE).

Sharding: data-parallel over the noise batch, 512 rows/core, 8 cores.
Each core runs the tiny MLP on its slice and writes a disjoint
[512,8192] f32 output slab.
"""

import numpy as np
import ml_dtypes

B = 4096
N = 8192
NF = 128
W = 32
NCORES = 8
BC = B // NCORES  # 512 rows per core

BF16 = ml_dtypes.bfloat16

_compiled = {}


def _hi_lo(x32):
    hi = x32.astype(BF16)
    lo = (x32 - hi.astype(np.float32)).astype(BF16)
    return hi, lo


def _build_program(reps=1):
    import concourse.bacc as bacc
    import concourse.tile as tile
    from concourse import mybir

    f32 = mybir.dt.float32
    bf16 = mybir.dt.bfloat16
    Tanh = mybir.ActivationFunctionType.Tanh

    nc = bacc.Bacc("TRN2", target_bir_lowering=False, debug=False,
                   enable_asserts=False, num_devices=NCORES)

    noiseT_d = nc.dram_tensor("noiseT", [NF, BC], f32, kind="ExternalInput").ap()
    w1_d = nc.dram_tensor("w1", [NF, W], f32, kind="ExternalInput").ap()
    w2_d = nc.dram_tensor("w2", [W, W], f32, kind="ExternalInput").ap()
    w3_d = nc.dram_tensor("w3", [W, W], f32, kind="ExternalInput").ap()
    w4_d = nc.dram_tensor("w4", [W, W], f32, kind="ExternalInput").ap()
    bmlp_d = nc.dram_tensor("bmlp", [W, 4], f32, kind="ExternalInput").ap()
    m1_d = nc.dram_tensor("m1", [W + 1, 3], f32, kind="ExternalInput").ap()
    rhs_d = nc.dram_tensor("rhs_aug", [8, N], bf16, kind="ExternalInput").ap()
    g_d = nc.dram_tensor("g", [BC, N], f32, kind="ExternalOutput").ap()

    import concourse.bass as bass

    with tile.TileContext(nc) as tc:
        with (
            tc.tile_pool(name="const", bufs=1) as cpool,
            tc.tile_pool(name="psum_prep", bufs=2, space="PSUM") as pp,
            tc.tile_pool(name="psum_main", bufs=4, space="PSUM") as pm,
            tc.tile_pool(name="gbuf", bufs=2) as gpool,
        ):
            def _body():
                _emit(nc, tc, cpool, pp, pm, gpool,
                      noiseT_d, w1_d, w2_d, w3_d, w4_d, bmlp_d, m1_d,
                      rhs_d, g_d, f32, bf16, Tanh, bass)

            if reps == 1:
                _body()
            else:
                with tc.For_i(0, reps):
                    _body()

    nc.compile()
    return nc


def _emit(nc, tc, cpool, pp, pm, gpool,
          noiseT_d, w1_d, w2_d, w3_d, w4_d, bmlp_d, m1_d,
          rhs_d, g_d, f32, bf16, Tanh, bass):
    if True:
        if True:
            noiseT = cpool.tile([NF, BC], f32)
            w1 = cpool.tile([NF, W], f32)
            w2 = cpool.tile([W, W], f32)
            w3 = cpool.tile([W, W], f32)
            w4 = cpool.tile([W, W], f32)
            bmlp = cpool.tile([W, 4], f32)
            m1 = cpool.tile([W + 1, 3], f32)
            rhs_aug = cpool.tile([8, N], bf16)

            nc.sync.dma_start(noiseT[:], noiseT_d[:])
            nc.sync.dma_start(w1[:], w1_d[:])
            nc.sync.dma_start(w2[:], w2_d[:])
            nc.sync.dma_start(w3[:], w3_d[:])
            nc.sync.dma_start(w4[:], w4_d[:])
            nc.sync.dma_start(bmlp[:], bmlp_d[:])
            nc.sync.dma_start(m1[:], m1_d[:])
            nc.gpsimd.dma_start(rhs_aug[:], rhs_d[:])

            h4aug = cpool.tile([W + 1, BC], f32)
            nc.gpsimd.memset(h4aug[W:W + 1, :], 1.0)

            h1 = cpool.tile([W, BC], f32)
            h2 = cpool.tile([W, BC], f32)
            h3 = cpool.tile([W, BC], f32)

            ps = pp.tile([W, BC], f32)
            nc.tensor.matmul(ps[:], w1[:], noiseT[:], start=True, stop=True)
            nc.scalar.activation(h1[:], ps[:], Tanh, bias=bmlp[:, 0:1])

            ps = pp.tile([W, BC], f32)
            nc.tensor.matmul(ps[:], w2[:], h1[:], start=True, stop=True)
            nc.scalar.activation(h2[:], ps[:], Tanh, bias=bmlp[:, 1:2])

            ps = pp.tile([W, BC], f32)
            nc.tensor.matmul(ps[:], w3[:], h2[:], start=True, stop=True)
            nc.scalar.activation(h3[:], ps[:], Tanh, bias=bmlp[:, 2:3])

            ps = pp.tile([W, BC], f32)
            nc.tensor.matmul(ps[:], w4[:], h3[:], start=True, stop=True)
            nc.scalar.activation(h4aug[0:W, :], ps[:], Tanh, bias=bmlp[:, 3:4])

            # Aq[3, BC] = M1.T @ [h4; 1]: rows a0, a1, q
            psA = pp.tile([3, BC], f32)
            nc.tensor.matmul(psA[:], m1[:], h4aug[:], start=True, stop=True)

            aq = cpool.tile([3, BC], f32)
            nc.vector.tensor_copy(aq[:], psA[:])

            # lhsT_aug rows: [a0h a1h qh a0l a1l ql a0h a1h]
            # compute hi/lo in partition-base-0 tiles (engine partition
            # access must be 32-aligned), then place rows by SBUF->SBUF
            # DMA which has no partition-base restriction.
            lhsT = cpool.tile([8, BC], bf16)
            hi_bf = cpool.tile([3, BC], bf16)
            lo_bf = cpool.tile([3, BC], bf16)
            hi32 = cpool.tile([3, BC], f32)
            lo32 = cpool.tile([3, BC], f32)
            nc.vector.tensor_copy(hi_bf[:], aq[:])
            nc.vector.tensor_copy(hi32[:], hi_bf[:])
            nc.vector.tensor_sub(lo32[:], aq[:], hi32[:])
            nc.vector.tensor_copy(lo_bf[:], lo32[:])
            nc.sync.dma_start(lhsT[0:3, :], hi_bf[:])
            nc.sync.dma_start(lhsT[3:6, :], lo_bf[:])
            nc.sync.dma_start(lhsT[6:8, :], hi_bf[0:2, :])

            # main loop: 4 row-blocks x 16 col-chunks of 512
            NB = BC // 128       # 4
            NCHUNK = N // 512    # 16
            for rb in range(NB):
                g_row = gpool.tile([128, N], f32)
                lt = lhsT[:, bass.ts(rb, 128)]
                for cc in range(NCHUNK):
                    psg = pm.tile([128, 512], f32)
                    nc.tensor.matmul(psg[:], lt, rhs_aug[:, bass.ts(cc, 512)],
                                     start=True, stop=True)
                    dst = g_row[:, bass.ts(cc, 512)]
                    if cc % 2 == 0:
                        nc.vector.tensor_copy(dst, psg[:])
                    else:
                        nc.scalar.copy(dst, psg[:])
                nc.sync.dma_start(g_d[bass.ts(rb, 128), :], g_row[:])


def _get_program(reps=1):
    if reps not in _compiled:
        _compiled[reps] = _build_program(reps)
    return _compiled[reps]


def kernel(**inputs):
    return run(inputs)[0]


def prepare_in_maps(inputs):
    noise = inputs["noise"].astype(np.float32, copy=False)
    coords = inputs["coordinates"].astype(np.float32, copy=False)
    W1 = inputs["W1"]; b1 = inputs["b1"]
    W2 = inputs["W2"]; b2 = inputs["b2"]
    W3 = inputs["W3"]; b3 = inputs["b3"]
    W4 = inputs["W4"]; b4 = inputs["b4"]
    W5 = inputs["W5"].astype(np.float64)
    b5 = inputs["b5"].astype(np.float64)
    Wc = inputs["Wc"].astype(np.float64)
    bc = inputs["bc"].astype(np.float64)
    Wm = inputs["Wm"].astype(np.float64)
    bm = inputs["bm"].astype(np.float64)

    # fold trunk + bias affine maps into M1 [33,3]
    M1 = np.zeros((W + 1, 3), np.float32)
    M1[:W, 0:2] = (W5 @ Wc.T).astype(np.float32)
    M1[W, 0:2] = (Wc @ b5 + Wm[:, 0]).astype(np.float32)
    M1[:W, 2] = (W5 @ bc).astype(np.float32)
    M1[W, 2] = np.float32(b5 @ bc + bm[0])

    bmlp = np.stack([b1, b2, b3, b4], axis=1).astype(np.float32)

    x, y = coords[:, 0], coords[:, 1]
    xh, xl = _hi_lo(x)
    yh, yl = _hi_lo(y)
    ones = np.ones(N, BF16)
    rhs_aug = np.ascontiguousarray(
        np.stack([xh, yh, ones, xh, yh, ones, xl, yl]))

    common = {
        "w1": np.ascontiguousarray(W1.astype(np.float32)),
        "w2": np.ascontiguousarray(W2.astype(np.float32)),
        "w3": np.ascontiguousarray(W3.astype(np.float32)),
        "w4": np.ascontiguousarray(W4.astype(np.float32)),
        "bmlp": bmlp,
        "m1": M1,
        "rhs_aug": rhs_aug,
    }
    in_maps = []
    for c in range(NCORES):
        m = dict(common)
        m["noiseT"] = np.ascontiguousarray(noise[c * BC:(c + 1) * BC].T)
        in_maps.append(m)
    return in_maps


def run(inputs):
    from concourse.bass_utils import run_bass_kernel_spmd

    in_maps = prepare_in_maps(inputs)
    nc = _get_program()
    res = run_bass_kernel_spmd(nc, in_maps, list(range(NCORES)))
    out = np.concatenate([res.results[c]["g"] for c in range(NCORES)], axis=0)
    return out.reshape(-1), getattr(res, "exec_time_ns", None)


# revision 11
# speedup vs baseline: 66156.8048x; 1.0104x over previous
"""PodDeepONet kernel for 8 Trainium2 NeuronCores.

Reference computes:
    h = tanh chain: noise[4096,128] -> W1..W4 (width 32) -> h4[4096,32]
    out_b  = h4 @ W5 + b5                      [4096,256]
    out_t  = coords @ Wc + bc                  [8192,256]
    out_bias = (coords @ Wm + bm)[:,0]         [8192]
    g = out_b @ out_t.T + out_bias[None,:]     [4096,8192] -> reshape(-1)

Since out_t / out_bias are affine in the 2-D coords, the K=256 GEMM
collapses algebraically:
    g[i,j] = a0[i]*x[j] + a1[i]*y[j] + q[i]
with   [a0,a1,q][i] = [h4[i],1] @ M1,   M1[33,3] folded on host from
W5,b5,Wc,bc,Wm,bm (float64).  The rank-3 product is evaluated on device
in bf16 with hi/lo error compensation split across two accumulating
matmuls per output tile:
    A (5 rows): [a0h,a1h,qh,a0h,a1h] . [xh,yh,1,xl,yl]
    B (3 rows): [a0l,a1l,ql]         . [xh,yh,1]
which covers all first-order rounding terms (~8e-6 rel error).

Sharding: data-parallel over the noise batch, 512 rows/core, 8 cores.
Each core runs the tiny MLP on its slice and writes a disjoint
[512,8192] f32 output slab.

Schedule: prep is pipelined per 128-row block and overlapped with the
main GEMM loop; all small inputs ride one packed [128,656] f32 DMA; the
output streams out in 1 MiB DMAs issued per quarter row-block.  The
kernel is HBM-write-bound (16 MiB/core at ~350 GB/s ~= 48 us).
"""

import numpy as np
import ml_dtypes

B = 4096
N = 8192
NF = 128
W = 32
NCORES = 8
BC = B // NCORES  # 512 rows per core
PACK_COLS = 656

BF16 = ml_dtypes.bfloat16

_compiled = {}


def _hi_lo(x32):
    hi = x32.astype(BF16)
    lo = (x32 - hi.astype(np.float32)).astype(BF16)
    return hi, lo


def _build_program(reps=1):
    import concourse.bacc as bacc
    import concourse.tile as tile
    from concourse import mybir

    f32 = mybir.dt.float32
    bf16 = mybir.dt.bfloat16
    Tanh = mybir.ActivationFunctionType.Tanh

    nc = bacc.Bacc("TRN2", target_bir_lowering=False, debug=False,
                   enable_asserts=False, num_devices=NCORES)

    packed_d = nc.dram_tensor("packed", [NF, PACK_COLS], f32,
                              kind="ExternalInput").ap()
    rhs5_d = nc.dram_tensor("rhs5", [5, N], bf16, kind="ExternalInput").ap()
    g_d = nc.dram_tensor("g", [BC, N], f32, kind="ExternalOutput").ap()

    import concourse.bass as bass

    with tile.TileContext(nc) as tc:
        with (
            tc.tile_pool(name="const", bufs=1) as cpool,
            tc.tile_pool(name="aux", bufs=2) as apool,
            tc.tile_pool(name="ps_mlp", bufs=2, space="PSUM") as pp,
            tc.tile_pool(name="ps_aq", bufs=2, space="PSUM") as pa,
            tc.tile_pool(name="ps_main", bufs=4, space="PSUM") as pm,
            tc.tile_pool(name="gbuf", bufs=2) as gpool,
        ):
            def _body():
                _emit(nc, tc, cpool, apool, pp, pa, pm, gpool,
                      packed_d, rhs5_d, g_d, f32, bf16, Tanh, bass)

            if reps == 1:
                _body()
            else:
                with tc.For_i(0, reps):
                    _body()

    nc.compile()
    return nc


def _emit(nc, tc, cpool, apool, pp, pa, pm, gpool,
          packed_d, rhs5_d, g_d, f32, bf16, Tanh, bass):
    packed = cpool.tile([NF, PACK_COLS], f32)
    rhs5 = cpool.tile([5, N], bf16)
    nc.sync.dma_start(packed[:], packed_d[:])
    nc.gpsimd.dma_start(rhs5[:], rhs5_d[:])

    noiseT = packed[:, 0:512]
    w1 = packed[:, 512:544]
    w2 = packed[0:W, 544:576]
    w3 = packed[0:W, 576:608]
    w4 = packed[0:W, 608:640]
    bmlp = packed[0:W, 640:644]
    m1a = packed[0:W + 1, 644:649]

    h4aug = cpool.tile([W + 1, BC], f32)
    nc.gpsimd.memset(h4aug[W:W + 1, :], 1.0)

    NB = BC // 128       # 4 row-blocks
    NCHUNK = N // 512    # 16 col-chunks
    for rb in range(NB):
        cols = bass.ts(rb, 128)
        # branch MLP on this row-block (tanh chain, width 32)
        h1 = apool.tile([W, 128], f32)
        h2 = apool.tile([W, 128], f32)
        h3 = apool.tile([W, 128], f32)
        ps = pp.tile([W, 128], f32)
        nc.tensor.matmul(ps[:], w1, noiseT[:, cols], start=True, stop=True)
        nc.scalar.activation(h1[:], ps[:], Tanh, bias=bmlp[:, 0:1])
        ps = pp.tile([W, 128], f32)
        nc.tensor.matmul(ps[:], w2, h1[:], start=True, stop=True)
        nc.scalar.activation(h2[:], ps[:], Tanh, bias=bmlp[:, 1:2])
        ps = pp.tile([W, 128], f32)
        nc.tensor.matmul(ps[:], w3, h2[:], start=True, stop=True)
        nc.scalar.activation(h3[:], ps[:], Tanh, bias=bmlp[:, 2:3])
        ps = pp.tile([W, 128], f32)
        nc.tensor.matmul(ps[:], w4, h3[:], start=True, stop=True)
        nc.scalar.activation(h4aug[0:W, cols], ps[:], Tanh, bias=bmlp[:, 3:4])

        # psA rows = [a0, a1, q, a0, a1] for this row-block
        psA = pa.tile([5, 128], f32)
        nc.tensor.matmul(psA[:], m1a, h4aug[:, cols], start=True, stop=True)

        # hi/lo split: PSUM reads must go via ACT/DVE (Pool cannot touch
        # PSUM); SBUF-only hops ride the Pool engine.
        hiA = apool.tile([5, 128], bf16)
        hiA32 = apool.tile([5, 128], f32)
        loB32 = apool.tile([3, 128], f32)
        loB = apool.tile([3, 128], bf16)
        nc.scalar.copy(hiA[:], psA[:])
        nc.gpsimd.tensor_copy(hiA32[:], hiA[:])
        nc.vector.tensor_sub(loB32[:], psA[0:3, :], hiA32[0:3, :])
        nc.gpsimd.tensor_copy(loB[:], loB32[:])

        g_row = gpool.tile([128, N], f32)
        for cc in range(NCHUNK):
            ccols = bass.ts(cc, 512)
            psg = pm.tile([128, 512], f32)
            nc.tensor.matmul(psg[:], hiA[:], rhs5[:, ccols],
                             start=True, stop=False)
            nc.tensor.matmul(psg[:], loB[:], rhs5[0:3, ccols],
                             start=False, stop=True)
            dst = g_row[:, ccols]
            if cc % 2 == 0:
                nc.vector.tensor_copy(dst, psg[:])
            else:
                nc.scalar.copy(dst, psg[:])
            if cc % 4 == 3:
                qcols = bass.ts(cc // 4, 2048)
                nc.sync.dma_start(g_d[bass.ts(rb, 128), qcols],
                                  g_row[:, qcols])


def _get_program(reps=1):
    if reps not in _compiled:
        _compiled[reps] = _build_program(reps)
    return _compiled[reps]


def kernel(**inputs):
    return run(inputs)[0]


def prepare_in_maps(inputs):
    noise = inputs["noise"].astype(np.float32, copy=False)
    coords = inputs["coordinates"].astype(np.float32, copy=False)
    W1 = inputs["W1"]; b1 = inputs["b1"]
    W2 = inputs["W2"]; b2 = inputs["b2"]
    W3 = inputs["W3"]; b3 = inputs["b3"]
    W4 = inputs["W4"]; b4 = inputs["b4"]
    W5 = inputs["W5"].astype(np.float64)
    b5 = inputs["b5"].astype(np.float64)
    Wc = inputs["Wc"].astype(np.float64)
    bc = inputs["bc"].astype(np.float64)
    Wm = inputs["Wm"].astype(np.float64)
    bm = inputs["bm"].astype(np.float64)

    # fold trunk + bias affine maps into M1 [33,3]
    M1 = np.zeros((W + 1, 3), np.float32)
    M1[:W, 0:2] = (W5 @ Wc.T).astype(np.float32)
    M1[W, 0:2] = (Wc @ b5 + Wm[:, 0]).astype(np.float32)
    M1[:W, 2] = (W5 @ bc).astype(np.float32)
    M1[W, 2] = np.float32(b5 @ bc + bm[0])
    M1A = M1[:, [0, 1, 2, 0, 1]]  # [33,5]

    bmlp = np.stack([b1, b2, b3, b4], axis=1).astype(np.float32)

    x, y = coords[:, 0], coords[:, 1]
    xh, xl = _hi_lo(x)
    yh, yl = _hi_lo(y)
    ones = np.ones(N, BF16)
    rhs5 = np.ascontiguousarray(np.stack([xh, yh, ones, xl, yl]))

    packed_common = np.zeros((NF, PACK_COLS), np.float32)
    packed_common[:, 512:544] = W1.astype(np.float32)
    packed_common[0:W, 544:576] = W2.astype(np.float32)
    packed_common[0:W, 576:608] = W3.astype(np.float32)
    packed_common[0:W, 608:640] = W4.astype(np.float32)
    packed_common[0:W, 640:644] = bmlp
    packed_common[0:W + 1, 644:649] = M1A

    in_maps = []
    for c in range(NCORES):
        packed = packed_common.copy()
        packed[:, 0:512] = noise[c * BC:(c + 1) * BC].T
        in_maps.append({"packed": packed, "rhs5": rhs5})
    return in_maps


def run(inputs):
    from concourse.bass_utils import run_bass_kernel_spmd

    in_maps = prepare_in_maps(inputs)
    nc = _get_program()
    res = run_bass_kernel_spmd(nc, in_maps, list(range(NCORES)))
    out = np.concatenate([res.results[c]["g"] for c in range(NCORES)], axis=0)
    return out.reshape(-1), getattr(res, "exec_time_ns", None)


# revision 17
# speedup vs baseline: 82766.6863x; 1.2511x over previous
"""PodDeepONet kernel for 8 Trainium2 NeuronCores.

Reference computes:
    h = tanh chain: noise[4096,128] -> W1..W4 (width 32) -> h4[4096,32]
    out_b  = h4 @ W5 + b5                      [4096,256]
    out_t  = coords @ Wc + bc                  [8192,256]
    out_bias = (coords @ Wm + bm)[:,0]         [8192]
    g = out_b @ out_t.T + out_bias[None,:]     [4096,8192] -> reshape(-1)

Since out_t / out_bias are affine in the 2-D coords, the K=256 GEMM
collapses algebraically:
    g[i,j] = a0[i]*x[j] + a1[i]*y[j] + q[i]
with   [a0,a1,q][i] = [h4[i],1] @ M1,   M1[33,3] folded on host from
W5,b5,Wc,bc,Wm,bm (float64).  The rank-3 product is evaluated on device
in bf16 with hi/lo error compensation in ONE matmul per output tile by
exploiting partition alignment: lhsT is a [35,128] tile with
    rows 0:5   = [a0h,a1h,qh,a0h,a1h]   (hi parts)
    rows 32:35 = [a0l,a1l,ql]           (lo parts, 32-aligned so compute
                                         engines can write them)
    rows 5:32  = zeros
matched against rhs rows [xh,yh,1,xl,yl, 0...0, xh,yh,1].  The zero
rows cost nothing: matmul time is set by the 512 moving columns, not K.
This covers all first-order rounding terms (~8e-6 rel error).

Sharding: data-parallel over the noise batch, 512 rows/core, 8 cores.
Each core runs the tiny MLP on its slice and writes a disjoint
[512,8192] f32 output slab.

Schedule: prep is pipelined per 128-row block and overlapped with the
main GEMM loop; all small inputs ride one packed [128,656] f32 DMA; the
output streams out in 1 MiB DMAs issued per quarter row-block.  The
kernel is HBM-write-bound (16 MiB/core at ~350 GB/s ~= 48 us).
"""

import numpy as np
import ml_dtypes

B = 4096
N = 8192
NF = 128
W = 32
NCORES = 8
BC = B // NCORES  # 512 rows per core
PACK_COLS = 656
KA = 35  # augmented K: hi rows 0:5, zero rows 5:32, lo rows 32:35

BF16 = ml_dtypes.bfloat16

_compiled = {}


def _hi_lo(x32):
    hi = x32.astype(BF16)
    lo = (x32 - hi.astype(np.float32)).astype(BF16)
    return hi, lo


def _build_program(reps=1):
    import concourse.bacc as bacc
    import concourse.tile as tile
    from concourse import mybir

    f32 = mybir.dt.float32
    bf16 = mybir.dt.bfloat16
    Tanh = mybir.ActivationFunctionType.Tanh

    nc = bacc.Bacc("TRN2", target_bir_lowering=False, debug=False,
                   enable_asserts=False, num_devices=NCORES)

    packed_d = nc.dram_tensor("packed", [NF, PACK_COLS], f32,
                              kind="ExternalInput").ap()
    rhs35_d = nc.dram_tensor("rhs35", [KA, N], bf16,
                             kind="ExternalInput").ap()
    g_d = nc.dram_tensor("g", [BC, N], f32, kind="ExternalOutput").ap()

    import concourse.bass as bass

    with tile.TileContext(nc) as tc:
        with (
            tc.tile_pool(name="const", bufs=1) as cpool,
            tc.tile_pool(name="aux", bufs=2) as apool,
            tc.tile_pool(name="ps_mlp", bufs=2, space="PSUM") as pp,
            tc.tile_pool(name="ps_aq", bufs=2, space="PSUM") as pa,
            tc.tile_pool(name="ps_main", bufs=4, space="PSUM") as pm,
            tc.tile_pool(name="gbuf", bufs=2) as gpool,
        ):
            def _body():
                _emit(nc, tc, cpool, apool, pp, pa, pm, gpool,
                      packed_d, rhs35_d, g_d, f32, bf16, Tanh, bass)

            if reps == 1:
                _body()
            else:
                with tc.For_i(0, reps):
                    _body()

    nc.compile()
    return nc


def _emit(nc, tc, cpool, apool, pp, pa, pm, gpool,
          packed_d, rhs35_d, g_d, f32, bf16, Tanh, bass):
    packed = cpool.tile([NF, PACK_COLS], f32)
    rhsA = cpool.tile([KA, N], bf16)
    lhsT = cpool.tile([KA, BC], bf16)
    nc.vector.memset(lhsT[:], 0.0)
    nc.sync.dma_start(packed[:], packed_d[:])
    nc.sync.dma_start(rhsA[:], rhs35_d[:])

    noiseT = packed[:, 0:512]
    w1 = packed[:, 512:544]
    w2 = packed[0:W, 544:576]
    w3 = packed[0:W, 576:608]
    w4 = packed[0:W, 608:640]
    bmlp = packed[0:W, 640:644]
    m1a = packed[0:W + 1, 644:649]

    h4aug = cpool.tile([W + 1, BC], f32)
    nc.gpsimd.memset(h4aug[W:W + 1, :], 1.0)

    NB = BC // 128       # 4 row-blocks
    NCHUNK = N // 512    # 16 col-chunks
    for rb in range(NB):
        cols = bass.ts(rb, 128)
        # branch MLP on this row-block (tanh chain, width 32)
        h1 = apool.tile([W, 128], f32)
        h2 = apool.tile([W, 128], f32)
        h3 = apool.tile([W, 128], f32)
        ps = pp.tile([W, 128], f32)
        nc.tensor.matmul(ps[:], w1, noiseT[:, cols], start=True, stop=True)
        nc.scalar.activation(h1[:], ps[:], Tanh, bias=bmlp[:, 0:1])
        ps = pp.tile([W, 128], f32)
        nc.tensor.matmul(ps[:], w2, h1[:], start=True, stop=True)
        nc.scalar.activation(h2[:], ps[:], Tanh, bias=bmlp[:, 1:2])
        ps = pp.tile([W, 128], f32)
        nc.tensor.matmul(ps[:], w3, h2[:], start=True, stop=True)
        nc.scalar.activation(h3[:], ps[:], Tanh, bias=bmlp[:, 2:3])
        ps = pp.tile([W, 128], f32)
        nc.tensor.matmul(ps[:], w4, h3[:], start=True, stop=True)
        nc.scalar.activation(h4aug[0:W, cols], ps[:], Tanh, bias=bmlp[:, 3:4])

        # psA rows = [a0, a1, q, a0, a1] for this row-block
        psA = pa.tile([5, 128], f32)
        nc.tensor.matmul(psA[:], m1a, h4aug[:, cols], start=True, stop=True)

        # hi rows into lhsT[0:5], lo rows into lhsT[32:35] (32-aligned).
        # PSUM reads must go via ACT/DVE; Pool handles SBUF-only hops.
        hi32 = apool.tile([5, 128], f32)
        lo32 = apool.tile([3, 128], f32)
        nc.scalar.copy(lhsT[0:5, cols], psA[:])
        nc.gpsimd.tensor_copy(hi32[:], lhsT[0:5, cols])
        nc.vector.tensor_sub(lo32[:], psA[0:3, :], hi32[0:3, :])
        nc.gpsimd.tensor_copy(lhsT[32:35, cols], lo32[:])

        g_row = gpool.tile([128, N], f32)
        for cc in range(NCHUNK):
            ccols = bass.ts(cc, 512)
            psg = pm.tile([128, 512], f32)
            nc.tensor.matmul(psg[:], lhsT[:, cols], rhsA[:, ccols],
                             start=True, stop=True)
            dst = g_row[:, ccols]
            if cc % 2 == 0:
                nc.vector.tensor_copy(dst, psg[:])
            else:
                nc.scalar.copy(dst, psg[:])
            if cc % 4 == 3:
                qcols = bass.ts(cc // 4, 2048)
                nc.sync.dma_start(g_d[bass.ts(rb, 128), qcols],
                                  g_row[:, qcols])


def _get_program(reps=1):
    if reps not in _compiled:
        _compiled[reps] = _build_program(reps)
    return _compiled[reps]


def kernel(**inputs):
    return run(inputs)[0]


def prepare_in_maps(inputs):
    noise = inputs["noise"].astype(np.float32, copy=False)
    coords = inputs["coordinates"].astype(np.float32, copy=False)
    W1 = inputs["W1"]; b1 = inputs["b1"]
    W2 = inputs["W2"]; b2 = inputs["b2"]
    W3 = inputs["W3"]; b3 = inputs["b3"]
    W4 = inputs["W4"]; b4 = inputs["b4"]
    W5 = inputs["W5"].astype(np.float64)
    b5 = inputs["b5"].astype(np.float64)
    Wc = inputs["Wc"].astype(np.float64)
    bc = inputs["bc"].astype(np.float64)
    Wm = inputs["Wm"].astype(np.float64)
    bm = inputs["bm"].astype(np.float64)

    # fold trunk + bias affine maps into M1 [33,3]
    M1 = np.zeros((W + 1, 3), np.float32)
    M1[:W, 0:2] = (W5 @ Wc.T).astype(np.float32)
    M1[W, 0:2] = (Wc @ b5 + Wm[:, 0]).astype(np.float32)
    M1[:W, 2] = (W5 @ bc).astype(np.float32)
    M1[W, 2] = np.float32(b5 @ bc + bm[0])
    M1A = M1[:, [0, 1, 2, 0, 1]]  # [33,5]

    bmlp = np.stack([b1, b2, b3, b4], axis=1).astype(np.float32)

    x, y = coords[:, 0], coords[:, 1]
    xh, xl = _hi_lo(x)
    yh, yl = _hi_lo(y)
    ones = np.ones(N, BF16)
    rhs35 = np.zeros((KA, N), BF16)
    rhs35[0:5] = np.stack([xh, yh, ones, xl, yl])
    rhs35[32:35] = rhs35[0:3]

    packed_common = np.zeros((NF, PACK_COLS), np.float32)
    packed_common[:, 512:544] = W1.astype(np.float32)
    packed_common[0:W, 544:576] = W2.astype(np.float32)
    packed_common[0:W, 576:608] = W3.astype(np.float32)
    packed_common[0:W, 608:640] = W4.astype(np.float32)
    packed_common[0:W, 640:644] = bmlp
    packed_common[0:W + 1, 644:649] = M1A

    in_maps = []
    for c in range(NCORES):
        packed = packed_common.copy()
        packed[:, 0:512] = noise[c * BC:(c + 1) * BC].T
        in_maps.append({"packed": packed, "rhs35": rhs35})
    return in_maps


def run(inputs):
    from concourse.bass_utils import run_bass_kernel_spmd

    in_maps = prepare_in_maps(inputs)
    nc = _get_program()
    res = run_bass_kernel_spmd(nc, in_maps, list(range(NCORES)))
    out = np.concatenate([res.results[c]["g"] for c in range(NCORES)], axis=0)
    return out.reshape(-1), getattr(res, "exec_time_ns", None)
